# revision 1
# baseline (speedup 1.0000x reference)
"""Trainium2 Bass kernel for nn_DendriticLayerSiLU_Template.

out = silu(g) * (x @ W.T), where per (token n, unit h):
  a[n,h,w] = sum_s x[n, w*64+s] * T[h, w*64+s]      (32 windows of size 64)
  p = softmax(|a| / tau), tau=1  (over w)
  g[n,h] = sum_w p[n,h,w] * a[n,h,w]

Strategy: 8-way data-parallel over N=4096 tokens (512/core), fp16 on-chip.
The gate's elementwise pipeline is the wall (ACT: PSUM drain + exp; DVE:
abs + a*e + reduction trees), so the kernel is software-pipelined at
half-unit (16-window) granularity with one slot of lookahead:

  slot k:  PE   einsum half k (16 windows, pair matmuls -> PSUM pairs)
           ACT  drain pairs of half k, exp(|a|) of half k
           DVE  mult+trees of half k-1 interleaved with abs of half k as
                drains land, plus the finished unit's tail every 2nd slot

lin = x@W.T runs as one PE burst per token-tile inside slots 1-4 (PE is
~30% busy) using a dedicated 2-bank PSUM tile; DMA streams inputs as
(xT_c, tT_c, wT_c) triplets so the first einsum starts within ~2us.
"""

import sys

if "/opt/trn_rl_repo" not in sys.path:
    sys.path.insert(0, "/opt/trn_rl_repo")

import numpy as np

import concourse.bass as bass
import concourse.tile as tile
from concourse import bacc, mybir
from concourse.bass_utils import run_bass_kernel_spmd

# Problem shapes (hardcoded per harness contract)
N_TOKENS = 4096
D = 2048          # in_features
H = 1024          # out_features
WIN = 64          # window size
NW = 32           # num windows
NCORES = 8
TOK = N_TOKENS // NCORES    # tokens per core = 512
NTT = TOK // 128            # token tiles per core = 4
NHC = H // 512              # h chunks = 2
KC = D // 128               # k chunks for linear = 16

F16 = mybir.dt.float16
F32 = mybir.dt.float32
U16 = mybir.dt.uint16


def _build_module():
    nc = bacc.Bacc("TRN2", target_bir_lowering=False, debug=False,
                   num_devices=NCORES)

    xT = nc.dram_tensor("xT", [D, TOK], F16, kind="ExternalInput").ap()
    wT = nc.dram_tensor("wT", [D, H], F16, kind="ExternalInput").ap()
    tT = nc.dram_tensor("tT", [D, H], F16, kind="ExternalInput").ap()
    out = nc.dram_tensor("out", [TOK, H], F32, kind="ExternalOutput").ap()

    with tile.TileContext(nc) as tc, nc.allow_low_precision(
        reason="fp16 gate pipeline by design"
    ):
        _body(tc, nc, xT, wT, tT, out)

    nc.compile()
    return nc


class _HalfSlot:
    """One 16-window half-unit: the nd tile plus unit bookkeeping."""

    def __init__(self, nd, ndh, half, tt, hc):
        self.nd = nd       # [128, 2(a|e), 16, 512] fp16
        self.ndh = ndh     # per-unit [128, 2(half), 2(num|den), 512]
        self.half = half
        self.tt = tt
        self.hc = hc


def _body(tc, nc, xT, wT, tT, out):
    from contextlib import ExitStack

    from concourse.dve_ops import (
        RECIPROCAL_APPROX_FAST, RECIP_APPROX_FAST_CONSTS)

    ctx = ExitStack()
    with ctx:
        weights = ctx.enter_context(tc.tile_pool(name="weights", bufs=1))
        nd_p = ctx.enter_context(tc.tile_pool(name="nd", bufs=3))
        smalls = ctx.enter_context(tc.tile_pool(name="smalls", bufs=2))
        outs_p = ctx.enter_context(tc.tile_pool(name="outs", bufs=2))
        ppool = ctx.enter_context(tc.tile_pool(name="apsum", bufs=3,
                                               space="PSUM"))
        lpool = ctx.enter_context(tc.tile_pool(name="lpsum", bufs=1,
                                               space="PSUM"))

        # ---- resident inputs (fp16, pre-transposed on host) ----
        # DMA in (xT_c, tT_c, wT_c) triplets: the slot-0 einsum needs only
        # (xT_c, tT_c) pairs first — the einsum slots need only those, so
        # the PE/drain ramp is paced at ~1.1us per chunk — then wT, which
        # is first needed by the lin burst in slot 2 (~35us in).
        xT_t, wT_t, tT_t = [], [], []
        for c in range(KC):
            xt = weights.tile([128, TOK], F16, name=f"xT{c}", tag=f"xT{c}")
            nc.sync.dma_start(out=xt[:], in_=xT[c * 128:(c + 1) * 128, :])
            xT_t.append(xt)
            tt_ = weights.tile([128, H], F16, name=f"tT{c}", tag=f"tT{c}")
            nc.sync.dma_start(out=tt_[:], in_=tT[c * 128:(c + 1) * 128, :])
            tT_t.append(tt_)
        for c in range(KC):
            wt = weights.tile([128, H], F16, name=f"wT{c}", tag=f"wT{c}")
            nc.sync.dma_start(out=wt[:], in_=wT[c * 128:(c + 1) * 128, :])
            wT_t.append(wt)

        lin_bf = [weights.tile([128, 2, 512], F16, name=f"lin{t}",
                               tag=f"lin{t}") for t in range(NTT)]

        # ---- helpers -----------------------------------------------------
        def emit_pe_drains(slot, prs=range(8)):
            """PE einsum pairs + ACT pair drains for half-slot `slot`."""
            nd, tt, hc, half = slot.nd, slot.tt, slot.hc, slot.half
            tok_sl = bass.ts(tt, 128)
            h_sl = bass.ts(hc, 512)
            for pr in prs:
                aps = ppool.tile([128, 2, 512], F32, tag="aps", name="aps")
                for i in range(2):
                    w = half * 16 + pr * 2 + i
                    ct, ro = w // 2, (w % 2) * WIN
                    nc.tensor.matmul(
                        aps[:, i, :],
                        lhsT=xT_t[ct][ro:ro + WIN, tok_sl],
                        rhs=tT_t[ct][ro:ro + WIN, h_sl],
                        start=True, stop=True,
                    )
                nc.scalar.copy(out=nd[:, 0, pr * 2:pr * 2 + 2, :],
                               in_=aps[:, :, :])

        def emit_abs(slot, grp):
            """|a| -> plane 1 for one 4-window group (DVE int16 4x mode)."""
            gs = slice(grp * 4, grp * 4 + 4)
            nd = slot.nd
            nc.vector.tensor_scalar(
                out=nd[:, 1, gs, :].bitcast(U16),
                in0=nd[:, 0, gs, :].bitcast(U16),
                scalar1=0x7FFF, scalar2=None,
                op0=mybir.AluOpType.bitwise_and,
            )

        def emit_exp(slot, q):
            """e = exp(|a|) in place on plane 1 (8-window ACT op)."""
            qs = slice(q * 8, (q + 1) * 8)
            nd = slot.nd
            nc.scalar.activation(
                out=nd[:, 1, qs, :], in_=nd[:, 1, qs, :],
                func=mybir.ActivationFunctionType.Exp,
            )

        def emit_mult(slot, q):
            """prod = a * e in-place on plane 0 (8-window tensor_tensor)."""
            qs = slice(q * 8, (q + 1) * 8)
            nd = slot.nd
            nc.vector.tensor_tensor(
                out=nd[:, 0, qs, :], in0=nd[:, 0, qs, :],
                in1=nd[:, 1, qs, :], op=mybir.AluOpType.mult,
            )

        def emit_tree(slot):
            """Per-plane pairwise tree; result -> ndh[:, half]."""
            nd = slot.nd
            n = 8
            while n >= 2:
                for p in range(2):
                    nc.vector.tensor_tensor(
                        out=nd[:, p, 0:n, :], in0=nd[:, p, 0:n, :],
                        in1=nd[:, p, n:2 * n, :], op=mybir.AluOpType.add,
                    )
                n //= 2
            for p in range(2):
                nc.vector.tensor_tensor(
                    out=slot.ndh[:, slot.half, p, :],
                    in0=nd[:, p, 0, :], in1=nd[:, p, 1, :],
                    op=mybir.AluOpType.add,
                )

        def emit_lin(t):
            """One token tile of lin = x @ W.T: PE burst + ACT drain."""
            tok_sl = bass.ts(t, 128)
            lps = lpool.tile([128, 2, 512], F32, tag="lps", name="lps")
            for hc in range(NHC):
                for k in range(KC):
                    nc.tensor.matmul(
                        lps[:, hc, :],
                        lhsT=xT_t[k][:, tok_sl],
                        rhs=wT_t[k][:, bass.ts(hc, 512)],
                        start=(k == 0), stop=(k == KC - 1),
                    )
            # drain pre-scaled by 0.5 (free on ACT) so the tail's
            # g/2 * lin product is a plain 2x tensor_tensor
            nc.scalar.mul(out=lin_bf[t][:], in_=lps[:, :, :], mul=0.5)

        def emit_tail(slot):
            """Merge halves, g = num/den, out = silu(g)*lin, DMA."""
            ndh, tt, hc = slot.ndh, slot.tt, slot.hc
            for p in range(2):
                nc.vector.tensor_tensor(
                    out=ndh[:, 0, p, :], in0=ndh[:, 0, p, :],
                    in1=ndh[:, 1, p, :], op=mybir.AluOpType.add)
            rcp = smalls.tile([128, 512], F16, tag="rcp")
            nc.vector._custom_dve(
                RECIPROCAL_APPROX_FAST, out=rcp[:], in0=ndh[:, 0, 1, :],
                **RECIP_APPROX_FAST_CONSTS)
            g = smalls.tile([128, 512], F16, tag="g")
            nc.vector.tensor_tensor(
                out=g[:], in0=ndh[:, 0, 0, :], in1=rcp[:],
                op=mybir.AluOpType.mult)
            # silu(g) = g * (1 + tanh(g/2)) / 2; tanh shares the exp ACT
            # table set -> no table switches.
            th = rcp  # recycle
            nc.scalar.activation(
                out=th[:], in_=g[:],
                func=mybir.ActivationFunctionType.Tanh, scale=0.5,
            )
            gl = ndh[:, 0, 0, :]  # recycle dead slot
            nc.vector.tensor_tensor(
                out=gl, in0=g[:], in1=lin_bf[tt][:, hc, :],
                op=mybir.AluOpType.mult)
            o = outs_p.tile([128, 512], F32, tag="o")
            nc.vector.scalar_tensor_tensor(
                out=o[:], in0=th[:], scalar=1.0, in1=gl,
                op0=mybir.AluOpType.add, op1=mybir.AluOpType.mult)
            nc.sync.dma_start(
                out=out[bass.ts(tt, 128), bass.ts(hc, 512)], in_=o[:])

        # ---- main loop: 16 half-slots, one slot of lookahead --------------
        prev = None        # _HalfSlot whose DVE mult/tree runs this slot
        ndh_cur = None
        slot_idx = 0
        for tt in range(NTT):
            for hc in range(NHC):
                for half in range(2):
                    if half == 0:
                        ndh_cur = smalls.tile([128, 2, 2, 512], F16,
                                              tag="ndh")
                    nd_tile = nd_p.tile([128, 2, 16, 512], F16, tag="nd",
                                        name="nd")
                    cur = _HalfSlot(nd_tile, ndh_cur, half, tt, hc)
                    if slot_idx == 0:
                        # pipeline-fill: no carried DVE work exists yet, so
                        # run slot 0 quarter-eagerly — exp/mult of windows
                        # 0-7 overlap the drains of windows 8-15
                        emit_pe_drains(cur, range(4))
                        emit_abs(cur, 0)
                        emit_abs(cur, 1)
                        emit_exp(cur, 0)
                        emit_mult(cur, 0)
                        emit_pe_drains(cur, range(4, 8))
                        emit_abs(cur, 2)
                        emit_abs(cur, 3)
                        emit_exp(cur, 1)
                        emit_mult(cur, 1)
                        cur.mults_done = True
                        prev = cur
                        slot_idx += 1
                        continue
                    emit_pe_drains(cur)
                    # DVE: prev's mults interleaved with cur's abs groups
                    if prev is not None and not getattr(prev, "mults_done",
                                                       False):
                        emit_mult(prev, 0)
                        emit_abs(cur, 0)
                        emit_abs(cur, 1)
                        emit_mult(prev, 1)
                        emit_abs(cur, 2)
                        emit_abs(cur, 3)
                    else:
                        for g_ in range(4):
                            emit_abs(cur, g_)
                    # ACT: exps for cur (after cur's abs in program order)
                    emit_exp(cur, 0)
                    emit_exp(cur, 1)
                    # lin bursts occupy PE/ACT slack in slots 2-5: late
                    # enough that wT has streamed in, and slot 2 emits
                    # lin(tt0) just before unit (tt0,hc0)'s tail uses it
                    if 2 <= slot_idx <= NTT + 1:
                        emit_lin(slot_idx - 2)
                    # DVE: prev's tree, then the finished unit's tail
                    if prev is not None:
                        emit_tree(prev)
                        if prev.half == 1:
                            emit_tail(prev)
                    prev = cur
                    slot_idx += 1

        # ---- pipeline flush ----
        emit_mult(prev, 0)
        emit_mult(prev, 1)
        emit_tree(prev)
        emit_tail(prev)


_NC_CACHE = None


def _get_module():
    global _NC_CACHE
    if _NC_CACHE is None:
        _NC_CACHE = _build_module()
    return _NC_CACHE


def kernel(x: np.ndarray, template_flat: np.ndarray,
           weights: np.ndarray) -> np.ndarray:
    nc = _get_module()

    xT = np.ascontiguousarray(x.T.astype(np.float16))           # [D, N]
    wT = np.ascontiguousarray(weights.T.astype(np.float16))     # [D, H]
    tT = np.ascontiguousarray(template_flat.T.astype(np.float16))

    in_maps = []
    for c in range(NCORES):
        in_maps.append({
            "xT": np.ascontiguousarray(xT[:, c * TOK:(c + 1) * TOK]),
            "wT": wT,
            "tT": tT,
        })
    res = run_bass_kernel_spmd(nc, in_maps, core_ids=list(range(NCORES)))
    return np.concatenate([res.results[c]["out"] for c in range(NCORES)],
                          axis=0).astype(np.float32)

